# revision 9
# baseline (speedup 1.0000x reference)
"""MetaPathEncoder (4x GraphConv + mean fusion) as a Bass/Tile SPMD kernel on 8 TRN2 cores.

Strategy (1D dst-node sharding, all 4 metapaths per core):
  - Each core owns 1250 output rows (10000/8). Edges are bucketed on host by
    (core, path, 128-row dst tile) and source-deduplicated per bucket.
  - GraphConv norm factorization: deg_out(src)^-1/2 is pre-multiplied into a
    per-path copy of the features (bf16, in HBM); deg_in(dst)^-1/2 * 1/4 is
    applied per dst row by the ACT engine when copying the SpMM result out of
    PSUM. The scatter matrix S[slot, dst_local] holds small-int edge counts,
    shipped as fp8e4 (exact for counts < 16) and fed to the PE directly as
    the stationary operand (no upconvert pass).
  - Gathers are issued as PREPARE_ONLY descriptor generation (SWDGE queues
    0-3, round-robin by path) with a lookahead window, then fired by
    trigger_dma when the destination tile frees: desc-gen runs ahead of
    buffer recycling instead of serializing with it.
  - Segment-sum via PE matmuls accumulating in fp32 PSUM:
    h[dst, :] = sum_b S_b.T @ X_b; ACT copies h out of PSUM with the
    per-dst-row scale; PE transposes h (identity matmul) to fi-on-partitions;
    16 accumulating matmuls apply the four 512x512 weights:
    out = sum_p h_p @ W_p + mean(b). The [1250, 512] fp32 shard is DMA'd out
    and the host concatenates the 8 shards.
  - Const loads are split across HWDGE queues: idx (gather dependency) first
    on the sync queue, everything else on the scalar queue.
"""
import sys

for _p in ("/opt/trn_rl_repo",):
    if _p not in sys.path:
        sys.path.insert(0, _p)

import numpy as np
import ml_dtypes

import concourse.bass as bass
import concourse.tile as tile
from concourse import bacc, mybir
from concourse.bass_utils import run_bass_kernel_spmd

BF16 = ml_dtypes.bfloat16
F8E4 = ml_dtypes.float8_e4m3fn

N_NODES = 10000
N_PATHS = 4
IN_DIM = 512
OUT_DIM = 512
NCORES = 8
ROWS_PER_CORE = N_NODES // NCORES  # 1250
NTILES = (ROWS_PER_CORE + 127) // 128  # 10 (last tile has 98 rows)
NCALLS = NTILES * N_PATHS  # 40 gather calls per core
LOOKAHEAD = 4  # one prep per queue ahead of its trigger

_program_cache: dict[tuple, object] = {}


def _build_program(Bc: tuple):
    """Build the SPMD Bass program; Bc[call] = gather blocks for call (t*4+p)."""
    if Bc in _program_cache:
        return _program_cache[Bc]

    TI = sum(Bc) * 8    # idx cols (int16, wrapped 16x, replicated 8x)
    TS = sum(Bc) * 128  # S cols (fp8e4)

    dt = mybir.dt
    nc = bacc.Bacc(
        "TRN2",
        target_bir_lowering=False,
        debug=False,
        num_devices=NCORES,
        num_swdge_queues=4,
        dynamic_dma_scratch_size=32768,
    )

    featd = [
        nc.dram_tensor(f"feat{p}", [N_NODES, IN_DIM], dt.bfloat16, kind="ExternalInput").ap()
        for p in range(N_PATHS)
    ]
    idxd = nc.dram_tensor("idx", [128, TI], dt.int16, kind="ExternalInput").ap()
    sd = nc.dram_tensor("smat", [128, TS], dt.float8e4, kind="ExternalInput").ap()
    wd = nc.dram_tensor("w", [128, 16 * OUT_DIM], dt.bfloat16, kind="ExternalInput").ap()
    bmd = nc.dram_tensor("bm", [128, OUT_DIM], dt.float32, kind="ExternalInput").ap()
    bsd = nc.dram_tensor("bscale", [128, NCALLS], dt.float32, kind="ExternalInput").ap()
    identd = nc.dram_tensor("identity", [128, 128], dt.bfloat16, kind="ExternalInput").ap()
    outd = nc.dram_tensor("out", [ROWS_PER_CORE, OUT_DIM], dt.float32, kind="ExternalOutput").ap()

    off_i = [0]
    off_s = [0]
    for b in Bc:
        off_i.append(off_i[-1] + b * 8)
        off_s.append(off_s[-1] + b * 128)

    with tile.TileContext(nc) as tc:
        with (
            tc.tile_pool(name="const", bufs=1) as cpool,
            tc.tile_pool(name="g", bufs=6) as gpool,
            tc.tile_pool(name="s8", bufs=6) as s8pool,
            tc.tile_pool(name="hsb", bufs=3) as hsb_pool,
            tc.tile_pool(name="htsb", bufs=3) as htsb_pool,
            tc.tile_pool(name="osb", bufs=2) as osb_pool,
            tc.tile_pool(name="hps", bufs=3, space="PSUM") as hps_pool,
            tc.tile_pool(name="htps", bufs=2, space="PSUM") as htps_pool,
            tc.tile_pool(name="ops", bufs=2, space="PSUM") as ops_pool,
        ):
            # idx first (gathers depend on it); first tile's columns, then rest
            idx_sb = cpool.tile([128, TI], dt.int16)
            split = off_i[N_PATHS]
            nc.sync.dma_start(idx_sb[:, :split], idxd[:, :split])
            nc.sync.dma_start(idx_sb[:, split:], idxd[:, split:])
            # remaining consts on the scalar HWDGE queue (parallel with idx)
            bs_sb = cpool.tile([128, NCALLS], dt.float32)
            nc.scalar.dma_start(bs_sb[:], bsd[:])
            ident = cpool.tile([128, 128], dt.bfloat16)
            nc.scalar.dma_start(ident[:], identd[:])
            w_sb = cpool.tile([128, 16 * OUT_DIM], dt.bfloat16)
            nc.scalar.dma_start(w_sb[:], wd[:])
            bm_sb = cpool.tile([128, OUT_DIM], dt.float32)
            nc.scalar.dma_start(bm_sb[:], bmd[:])

            dma_sems = [nc.alloc_semaphore(f"gsem{c}") for c in range(NCALLS)]
            # sems are not cleared by allocation; clear before first use
            for s in dma_sems:
                nc.sync.sem_clear(s)

            g_tiles: dict[int, bass.AP] = {}
            s_tiles: dict[int, bass.AP] = {}
            out_ps = None

            def emit_prep(call: int):
                p = call % N_PATHS
                B = Bc[call]
                g = gpool.tile([128, B, IN_DIM], dt.bfloat16, name="g")
                g_tiles[call] = g
                nc.gpsimd.dma_gather(
                    g[:],
                    featd[p][:],
                    idx_sb[:, off_i[call] : off_i[call + 1]],
                    B * 128,
                    B * 128,
                    IN_DIM,
                    single_packet=False,
                    queue_num=p,
                    prepare_only=True,
                    sem=dma_sems[call],
                )
                s8 = s8pool.tile([128, B, 128], dt.float8e4, name="s8")
                s_tiles[call] = s8
                nc.sync.dma_start(
                    s8[:].rearrange("p b d -> p (b d)"),
                    sd[:, off_s[call] : off_s[call + 1]],
                )

            def emit_compute(call: int):
                nonlocal out_ps
                t, p = call // N_PATHS, call % N_PATHS
                B = Bc[call]
                nc.gpsimd.trigger_dma(count=None, queue_num=p)
                g = g_tiles.pop(call)
                s8 = s_tiles.pop(call)
                # explicit data-landed wait: each gather bumps its queue's sem
                # by 16 on completion; call is the (t+1)-th on queue p
                nc.tensor.wait_ge(dma_sems[call], 16)
                if p == 0:
                    out_ps = ops_pool.tile([128, OUT_DIM], dt.float32, name="out_ps")
                hp = hps_pool.tile([128, IN_DIM], dt.float32, name="hp")
                for bb in range(B):
                    nc.tensor.matmul(
                        hp[:],
                        s8[:, bb, :],
                        g[:, bb, :],
                        start=(bb == 0),
                        stop=(bb == B - 1),
                    )
                hs = hsb_pool.tile([128, IN_DIM], dt.bfloat16, name="hs")
                nc.scalar.mul(hs[:], hp[:], bs_sb[:, call : call + 1])
                htp = htps_pool.tile([128, IN_DIM], dt.bfloat16, name="htp")
                for cc in range(4):
                    nc.tensor.transpose(
                        htp[:, cc * 128 : (cc + 1) * 128],
                        hs[:, cc * 128 : (cc + 1) * 128],
                        ident[:],
                    )
                hts = htsb_pool.tile([128, IN_DIM], dt.bfloat16, name="hts")
                nc.vector.tensor_copy(hts[:], htp[:])
                for cc in range(4):
                    nc.tensor.matmul(
                        out_ps[:],
                        hts[:, cc * 128 : (cc + 1) * 128],
                        w_sb[:, (p * 4 + cc) * OUT_DIM : (p * 4 + cc + 1) * OUT_DIM],
                        start=(p == 0 and cc == 0),
                        stop=(p == N_PATHS - 1 and cc == 3),
                    )
                if p == N_PATHS - 1:
                    os_ = osb_pool.tile([128, OUT_DIM], dt.float32, name="os_")
                    nc.vector.tensor_add(os_[:], out_ps[:], bm_sb[:])
                    rows = min(128, ROWS_PER_CORE - t * 128)
                    nc.sync.dma_start(outd[t * 128 : t * 128 + rows, :], os_[:rows, :])

            for call in range(NCALLS + LOOKAHEAD):
                if call >= LOOKAHEAD:
                    emit_compute(call - LOOKAHEAD)
                if call < NCALLS:
                    emit_prep(call)

    nc.compile()
    _program_cache[Bc] = nc
    return nc


def _prep_host(feat, src, dst, W, b):
    """Host-side bucketing, dedup, factored norms, and fp8 S materialization.

    Returns (Bc tuple, shared dict, per-core dicts)."""
    src = np.asarray(src).astype(np.int64)
    dst = np.asarray(dst).astype(np.int64)
    feat = np.asarray(feat, dtype=np.float32)
    W = np.asarray(W, dtype=np.float32)
    b = np.asarray(b, dtype=np.float32)

    # weights laid out [fi_local(128), p*4+chunk, fo] for direct SBUF residence
    Wt = np.empty((128, 16, OUT_DIM), dtype=BF16)
    for p in range(N_PATHS):
        for c in range(4):
            Wt[:, p * 4 + c, :] = W[p, c * 128 : (c + 1) * 128, :].astype(BF16)
    Wt = np.ascontiguousarray(Wt.reshape(128, 16 * OUT_DIM))

    bmean = b.mean(0).astype(np.float32)
    bm_bcast = np.ascontiguousarray(np.broadcast_to(bmean, (128, OUT_DIM)))

    # factored norms: feat_p = feat * deg_out_p^-1/2 (bf16 in HBM);
    # bscale[dst] = deg_in_p(dst)^-1/2 * 1/4 applied post-SpMM on ACT
    feats = {}
    deg_ins = []
    sorted_data = []
    for p in range(N_PATHS):
        s, d = src[p], dst[p]
        deg_out = np.maximum(np.bincount(s, minlength=N_NODES), 1).astype(np.float64)
        deg_in = np.maximum(np.bincount(d, minlength=N_NODES), 1).astype(np.float64)
        feats[f"feat{p}"] = (feat * (deg_out**-0.5)[:, None]).astype(BF16)
        deg_ins.append(deg_in)
        order = np.argsort(d, kind="stable")
        sorted_data.append((s[order], d[order]))

    bounds = []
    for c in range(NCORES):
        base = c * ROWS_PER_CORE
        for t in range(NTILES):
            lo = base + t * 128
            hi = base + min((t + 1) * 128, ROWS_PER_CORE)
            bounds.append((lo, hi))
    los = np.array([lo for lo, _ in bounds])
    his = np.array([hi for _, hi in bounds])

    ranges = []
    for p in range(N_PATHS):
        ds = sorted_data[p][1]
        a = np.searchsorted(ds, los, side="left")
        e = np.searchsorted(ds, his, side="left")
        ranges.append((a, e))

    # dedup per (core, path, tile); Bc[call] = max over cores
    buckets = {}  # (c, call) -> (uniq_idx, S_u8 [U, 128])
    Bc = np.zeros(NCALLS, dtype=np.int64)
    for c in range(NCORES):
        for t in range(NTILES):
            lo = c * ROWS_PER_CORE + t * 128
            for p in range(N_PATHS):
                call = t * N_PATHS + p
                a, e = ranges[p][0][c * NTILES + t], ranges[p][1][c * NTILES + t]
                ss = sorted_data[p][0][a:e]
                dl = (sorted_data[p][1][a:e] - lo).astype(np.int64)
                uniq, inv = np.unique(ss, return_inverse=True)
                U = len(uniq)
                S = np.zeros((U, 128), dtype=np.int64)
                np.add.at(S, (inv, dl), 1)
                assert S.max() < 16  # exact in fp8e4
                buckets[(c, call)] = (uniq, S.astype(np.uint8))
                Bc[call] = max(Bc[call], (U + 127) // 128)
    Bc = np.maximum(Bc, 1)

    off_i = np.concatenate([[0], np.cumsum(Bc * 8)])
    off_s = np.concatenate([[0], np.cumsum(Bc * 128)])
    TI, TS = int(off_i[-1]), int(off_s[-1])

    per_core = []
    for c in range(NCORES):
        idxw = np.zeros((128, TI), dtype=np.int16)
        s_cols = np.zeros((128, TS), dtype=np.uint8)
        bsc = np.zeros((128, NCALLS), dtype=np.float32)
        for call in range(NCALLS):
            B = int(Bc[call])
            t, p = call // N_PATHS, call % N_PATHS
            uniq, S = buckets[(c, call)]
            U = len(uniq)
            idx_pad = np.zeros(B * 128, dtype=np.int16)
            idx_pad[:U] = uniq
            # dma_gather wrapped index layout: position j -> [j%16, j//16],
            # replicated across the 8 groups of 16 partitions
            w16 = idx_pad.reshape(B * 8, 16).T  # [16, B*8]
            idxw[:, off_i[call] : off_i[call + 1]] = np.tile(w16, (8, 1))
            S_pad = np.zeros((B * 128, 128), dtype=np.uint8)
            S_pad[:U] = S
            s_cols[:, off_s[call] : off_s[call + 1]] = (
                S_pad.reshape(B, 128, 128).transpose(1, 0, 2).reshape(128, B * 128)
            )
            lo = c * ROWS_PER_CORE + t * 128
            rows = min(128, ROWS_PER_CORE - t * 128)
            bsc[:rows, call] = (deg_ins[p][lo : lo + rows] ** -0.5) * 0.25
        per_core.append({
            "idx": idxw,
            "smat": s_cols.astype(np.float32).astype(F8E4),
            "bscale": bsc,
        })

    shared = {
        **feats,
        "w": Wt,
        "bm": bm_bcast,
        "identity": np.eye(128, dtype=BF16),
    }
    return tuple(int(x) for x in Bc), shared, per_core


def kernel(feat, src, dst, W, b):
    Bc, shared, per_core = _prep_host(feat, src, dst, W, b)
    nc = _build_program(Bc)
    in_maps = [{**shared, **pc} for pc in per_core]
    res = run_bass_kernel_spmd(nc, in_maps, list(range(NCORES)))
    out = np.concatenate([res.results[c]["out"] for c in range(NCORES)], axis=0)
    return out.astype(np.float32)


if __name__ == "__main__":
    rng = np.random.default_rng(0)
    feat = rng.standard_normal((N_NODES, IN_DIM), dtype=np.float32)
    src = rng.integers(0, N_NODES, (N_PATHS, 160000)).astype(np.int64)
    dst = rng.integers(0, N_NODES, (N_PATHS, 160000)).astype(np.int64)
    W = (rng.standard_normal((N_PATHS, IN_DIM, OUT_DIM), dtype=np.float32) / np.sqrt(IN_DIM)).astype(np.float32)
    b = np.zeros((N_PATHS, OUT_DIM), np.float32)
    out = kernel(feat=feat, src=src, dst=dst, W=W, b=b)
    print("kernel ran, out shape", out.shape, out.dtype)


# revision 12
# speedup vs baseline: 1.1186x; 1.1186x over previous
"""MetaPathEncoder (4x GraphConv + mean fusion) as a Bass/Tile SPMD kernel on 8 TRN2 cores.

Strategy (1D dst-node sharding, all 4 metapaths per core):
  - Each core owns 1250 output rows (10000/8), processed as 5 pairs of
    128-row dst tiles. Edges are bucketed on host by (core, path, tile-PAIR)
    and source-deduplicated across the pair: sources feeding both tiles of a
    pair are gathered once (~10% fewer rows than per-tile dedup). Slots are
    laid out [even-only | shared | odd-only] so each tile's scatter matrix
    covers a static contiguous block window.
  - GraphConv norm factorization: deg_out(src)^-1/2 is pre-multiplied into a
    per-path copy of the features (bf16, in HBM); deg_in(dst)^-1/2 * 1/4 is
    applied per dst row by the ACT engine when copying the SpMM result out of
    PSUM. The scatter matrix S[slot, dst_local] holds small-int edge counts,
    shipped as fp8e4 (exact for counts < 16) and fed to the PE directly as
    the stationary operand (no upconvert pass).
  - On device, per (pair, path): dma_gather the slot source rows (bf16) of
    feat_p from HBM, round-robin on SWDGE queues 0-3 by path.
  - Segment-sum via PE matmuls accumulating in fp32 PSUM per tile:
    h[dst, :] = sum_b S_b.T @ X_b over the tile's block window; ACT copies h
    out of PSUM with the per-dst-row scale; PE transposes h (identity matmul)
    to fi-on-partitions; 16 accumulating matmuls per tile apply the four
    512x512 weights: out = sum_p h_p @ W_p + mean(b). The [1250, 512] fp32
    shard is DMA'd out and the host concatenates the 8 shards.
  - Const loads are split across HWDGE queues: idx (gather dependency) first
    on the sync queue, everything else on the scalar queue.
"""
import sys

for _p in ("/opt/trn_rl_repo",):
    if _p not in sys.path:
        sys.path.insert(0, _p)

import numpy as np
import ml_dtypes

import concourse.bass as bass
import concourse.tile as tile
from concourse import bacc, mybir
from concourse.bass_utils import run_bass_kernel_spmd

BF16 = ml_dtypes.bfloat16
F8E4 = ml_dtypes.float8_e4m3fn

N_NODES = 10000
N_PATHS = 4
IN_DIM = 512
OUT_DIM = 512
NCORES = 8
ROWS_PER_CORE = N_NODES // NCORES  # 1250
NTILES = (ROWS_PER_CORE + 127) // 128  # 10 (last tile has 98 rows)
NPAIRS = NTILES // 2  # 5
NCALLS = NPAIRS * N_PATHS  # 20 gather calls per core
NTCALLS = NTILES * N_PATHS  # 40 per-tile compute units

_program_cache: dict[tuple, object] = {}


def _build_program(cfg: tuple):
    """Build the SPMD Bass program.

    cfg = (Bc, We, Wo): per call (pair*4+path): gather blocks, even-tile
    window blocks (from 0), odd-tile window blocks (ending at Bc)."""
    if cfg in _program_cache:
        return _program_cache[cfg]
    Bc, We, Wo = cfg

    TI = sum(Bc) * 8    # idx cols (int16, wrapped 16x, replicated 8x)
    TS = (sum(We) + sum(Wo)) * 128  # S cols (fp8e4), per-tile matrices

    dt = mybir.dt
    nc = bacc.Bacc(
        "TRN2",
        target_bir_lowering=False,
        debug=False,
        num_devices=NCORES,
        num_swdge_queues=4,
    )

    featd = [
        nc.dram_tensor(f"feat{p}", [N_NODES, IN_DIM], dt.bfloat16, kind="ExternalInput").ap()
        for p in range(N_PATHS)
    ]
    idxd = nc.dram_tensor("idx", [128, TI], dt.int16, kind="ExternalInput").ap()
    sd = nc.dram_tensor("smat", [128, TS], dt.float8e4, kind="ExternalInput").ap()
    wd = nc.dram_tensor("w", [128, 16 * OUT_DIM], dt.bfloat16, kind="ExternalInput").ap()
    bmd = nc.dram_tensor("bm", [128, OUT_DIM], dt.float32, kind="ExternalInput").ap()
    bsd = nc.dram_tensor("bscale", [128, NTCALLS], dt.float32, kind="ExternalInput").ap()
    identd = nc.dram_tensor("identity", [128, 128], dt.bfloat16, kind="ExternalInput").ap()
    outd = nc.dram_tensor("out", [ROWS_PER_CORE, OUT_DIM], dt.float32, kind="ExternalOutput").ap()

    off_i = [0]
    for bq in Bc:
        off_i.append(off_i[-1] + bq * 8)
    # S offsets: per (call, half)
    off_s = [0]
    for c in range(NCALLS):
        off_s.append(off_s[-1] + We[c] * 128)
        off_s.append(off_s[-1] + Wo[c] * 128)

    with tile.TileContext(nc) as tc:
        with (
            tc.tile_pool(name="const", bufs=1) as cpool,
            tc.tile_pool(name="g", bufs=4) as gpool,
            tc.tile_pool(name="s8", bufs=6) as s8pool,
            tc.tile_pool(name="hsb", bufs=3) as hsb_pool,
            tc.tile_pool(name="htsb", bufs=3) as htsb_pool,
            tc.tile_pool(name="osb", bufs=2) as osb_pool,
            tc.tile_pool(name="hps", bufs=2, space="PSUM") as hps_pool,
            tc.tile_pool(name="htps", bufs=2, space="PSUM") as htps_pool,
            tc.tile_pool(name="ops", bufs=2, space="PSUM") as ops_pool,
        ):
            # idx first (gathers depend on it); first pair's columns, then rest
            idx_sb = cpool.tile([128, TI], dt.int16)
            split = off_i[N_PATHS]
            nc.sync.dma_start(idx_sb[:, :split], idxd[:, :split])
            nc.sync.dma_start(idx_sb[:, split:], idxd[:, split:])
            # remaining consts on the scalar HWDGE queue (parallel with idx)
            bs_sb = cpool.tile([128, NTCALLS], dt.float32)
            nc.scalar.dma_start(bs_sb[:], bsd[:])
            ident = cpool.tile([128, 128], dt.bfloat16)
            nc.scalar.dma_start(ident[:], identd[:])
            w_sb = cpool.tile([128, 16 * OUT_DIM], dt.bfloat16)
            nc.scalar.dma_start(w_sb[:], wd[:])
            bm_sb = cpool.tile([128, OUT_DIM], dt.float32)
            nc.scalar.dma_start(bm_sb[:], bmd[:])

            for pair in range(NPAIRS):
                out_pse = ops_pool.tile([128, OUT_DIM], dt.float32, name="out_pse")
                out_pso = ops_pool.tile([128, OUT_DIM], dt.float32, name="out_pso")
                for p in range(N_PATHS):
                    call = pair * N_PATHS + p
                    B = Bc[call]
                    g = gpool.tile([128, B, IN_DIM], dt.bfloat16, name="g")
                    nc.gpsimd.dma_gather(
                        g[:],
                        featd[p][:],
                        idx_sb[:, off_i[call] : off_i[call + 1]],
                        B * 128,
                        B * 128,
                        IN_DIM,
                        single_packet=False,
                        queue_num=p,
                    )
                    for half, out_ps in ((0, out_pse), (1, out_pso)):
                        t = pair * 2 + half
                        W_t = We[call] if half == 0 else Wo[call]
                        blo = 0 if half == 0 else B - W_t
                        si = off_s[call * 2 + half]
                        s8 = s8pool.tile([128, W_t, 128], dt.float8e4, name="s8")
                        nc.sync.dma_start(
                            s8[:].rearrange("p b d -> p (b d)"),
                            sd[:, si : si + W_t * 128],
                        )
                        hp = hps_pool.tile([128, IN_DIM], dt.float32, name="hp")
                        for bb in range(W_t):
                            nc.tensor.matmul(
                                hp[:],
                                s8[:, bb, :],
                                g[:, blo + bb, :],
                                start=(bb == 0),
                                stop=(bb == W_t - 1),
                            )
                        tci = t * N_PATHS + p
                        hs = hsb_pool.tile([128, IN_DIM], dt.bfloat16, name="hs")
                        nc.scalar.mul(hs[:], hp[:], bs_sb[:, tci : tci + 1])
                        htp = htps_pool.tile([128, IN_DIM], dt.bfloat16, name="htp")
                        for cc in range(4):
                            nc.tensor.transpose(
                                htp[:, cc * 128 : (cc + 1) * 128],
                                hs[:, cc * 128 : (cc + 1) * 128],
                                ident[:],
                            )
                        hts = htsb_pool.tile([128, IN_DIM], dt.bfloat16, name="hts")
                        nc.vector.tensor_copy(hts[:], htp[:])
                        for cc in range(4):
                            nc.tensor.matmul(
                                out_ps[:],
                                hts[:, cc * 128 : (cc + 1) * 128],
                                w_sb[:, (p * 4 + cc) * OUT_DIM : (p * 4 + cc + 1) * OUT_DIM],
                                start=(p == 0 and cc == 0),
                                stop=(p == N_PATHS - 1 and cc == 3),
                            )
                for half, out_ps in ((0, out_pse), (1, out_pso)):
                    t = pair * 2 + half
                    os_ = osb_pool.tile([128, OUT_DIM], dt.float32, name="os_")
                    nc.vector.tensor_add(os_[:], out_ps[:], bm_sb[:])
                    rows = min(128, ROWS_PER_CORE - t * 128)
                    nc.sync.dma_start(outd[t * 128 : t * 128 + rows, :], os_[:rows, :])

    nc.compile()
    _program_cache[cfg] = nc
    return nc


def _prep_host(feat, src, dst, W, b):
    """Host-side pair bucketing, cross-tile dedup, windowed slot layout.

    Returns (cfg tuple, shared dict, per-core dicts)."""
    src = np.asarray(src).astype(np.int64)
    dst = np.asarray(dst).astype(np.int64)
    feat = np.asarray(feat, dtype=np.float32)
    W = np.asarray(W, dtype=np.float32)
    b = np.asarray(b, dtype=np.float32)

    Wt = np.empty((128, 16, OUT_DIM), dtype=BF16)
    for p in range(N_PATHS):
        for c in range(4):
            Wt[:, p * 4 + c, :] = W[p, c * 128 : (c + 1) * 128, :].astype(BF16)
    Wt = np.ascontiguousarray(Wt.reshape(128, 16 * OUT_DIM))

    bmean = b.mean(0).astype(np.float32)
    bm_bcast = np.ascontiguousarray(np.broadcast_to(bmean, (128, OUT_DIM)))

    feats = {}
    deg_ins = []
    sorted_data = []
    for p in range(N_PATHS):
        s, d = src[p], dst[p]
        deg_out = np.maximum(np.bincount(s, minlength=N_NODES), 1).astype(np.float64)
        deg_in = np.maximum(np.bincount(d, minlength=N_NODES), 1).astype(np.float64)
        feats[f"feat{p}"] = (feat * (deg_out**-0.5)[:, None]).astype(BF16)
        deg_ins.append(deg_in)
        order = np.argsort(d, kind="stable")
        sorted_data.append((s[order], d[order]))

    # pair bounds: rows [pair*256, min(pair*256+256, 1250)) per core
    los, his = [], []
    for c in range(NCORES):
        base = c * ROWS_PER_CORE
        for pr in range(NPAIRS):
            los.append(base + pr * 256)
            his.append(base + min((pr + 1) * 256, ROWS_PER_CORE))
    los, his = np.array(los), np.array(his)
    ranges = []
    for p in range(N_PATHS):
        ds = sorted_data[p][1]
        a = np.searchsorted(ds, los, side="left")
        e = np.searchsorted(ds, his, side="left")
        ranges.append((a, e))

    # per (core, pair, path): slots = unique sources of the pair;
    # categorize even-only / shared / odd-only
    buckets = {}
    n_cat = {}
    Bc = np.zeros(NCALLS, dtype=np.int64)
    for c in range(NCORES):
        for pr in range(NPAIRS):
            lo = c * ROWS_PER_CORE + pr * 256
            for p in range(N_PATHS):
                call = pr * N_PATHS + p
                a, e = ranges[p][0][c * NPAIRS + pr], ranges[p][1][c * NPAIRS + pr]
                ss = sorted_data[p][0][a:e]
                dl = (sorted_data[p][1][a:e] - lo).astype(np.int64)  # 0..255
                half = dl // 128
                uniq, inv = np.unique(ss, return_inverse=True)
                U = len(uniq)
                in_e = np.zeros(U, bool)
                in_o = np.zeros(U, bool)
                in_e[inv[half == 0]] = True
                in_o[inv[half == 1]] = True
                cat = np.where(in_e & in_o, 1, np.where(in_e, 0, 2))
                order2 = np.argsort(cat, kind="stable")
                uniq_s = uniq[order2]
                rank = np.empty(U, np.int64)
                rank[order2] = np.arange(U)
                slot_of_edge = rank[inv]
                n_e = int((cat == 0).sum())
                n_sh = int((cat == 1).sum())
                n_o = int((cat == 2).sum())
                buckets[(c, call)] = (uniq_s, slot_of_edge, dl, half, n_e, n_sh, n_o)
                n_cat[(c, call)] = (n_e, n_sh, n_o)
                Bc[call] = max(Bc[call], (U + 127) // 128)
    Bc = np.maximum(Bc, 1)

    # static windows: even tile covers blocks [0, We); odd covers [B-Wo, B)
    We = np.zeros(NCALLS, dtype=np.int64)
    Wo = np.zeros(NCALLS, dtype=np.int64)
    for call in range(NCALLS):
        B = int(Bc[call])
        for c in range(NCORES):
            n_e, n_sh, n_o = n_cat[(c, call)]
            We[call] = max(We[call], (n_e + n_sh + 127) // 128)
            Wo[call] = max(Wo[call], (n_sh + n_o + 127) // 128)
        We[call] = max(1, min(int(We[call]), B))
        Wo[call] = max(1, min(int(Wo[call]), B))
        # the window overlap [B-Wo, We) must hold every core's shared slots
        need = max((n_cat[(c, call)][1] + 127) // 128 for c in range(NCORES))
        if int(We[call]) + int(Wo[call]) - B < need:
            We[call] = min(B, B + need - int(Wo[call]))

    off_i = np.concatenate([[0], np.cumsum(Bc * 8)])
    off_s = [0]
    for call in range(NCALLS):
        off_s.append(off_s[-1] + int(We[call]) * 128)
        off_s.append(off_s[-1] + int(Wo[call]) * 128)
    TI, TS = int(off_i[-1]), int(off_s[-1])

    per_core = []
    for c in range(NCORES):
        idxw = np.zeros((128, TI), dtype=np.int16)
        s_cols = np.zeros((128, TS), dtype=np.uint8)
        bsc = np.zeros((128, NTCALLS), dtype=np.float32)
        for call in range(NCALLS):
            B = int(Bc[call])
            pr, p = call // N_PATHS, call % N_PATHS
            uniq_s, slot_of_edge, dl, half, n_e, n_sh, n_o = buckets[(c, call)]
            U = len(uniq_s)
            # place slots: even-only at 0; shared inside both windows;
            # odd-only at the end. Shared must sit in [B-Wo, We)*128.
            sh_lo_min = (B - int(Wo[call])) * 128
            sh_start = max(n_e, sh_lo_min)
            assert sh_start + n_sh <= int(We[call]) * 128, (call, c)
            o_start = B * 128 - n_o
            assert o_start >= sh_start + n_sh, (call, c)
            pos = np.empty(U, np.int64)
            pos[:n_e] = np.arange(n_e)
            pos[n_e : n_e + n_sh] = sh_start + np.arange(n_sh)
            pos[n_e + n_sh :] = o_start + np.arange(n_o)
            idx_pad = np.zeros(B * 128, dtype=np.int16)
            idx_pad[pos] = uniq_s
            w16 = idx_pad.reshape(B * 8, 16).T
            idxw[:, off_i[call] : off_i[call + 1]] = np.tile(w16, (8, 1))
            # per-tile S over its window
            slot_pos_of_edge = pos[slot_of_edge]
            for hf in range(2):
                W_t = int(We[call]) if hf == 0 else int(Wo[call])
                blo = 0 if hf == 0 else B - W_t
                m = half == hf
                sp = slot_pos_of_edge[m] - blo * 128
                dloc = dl[m] - hf * 128
                S = np.zeros((W_t * 128, 128), dtype=np.int64)
                np.add.at(S, (sp, dloc), 1)
                assert S.max() < 16
                si = off_s[call * 2 + hf]
                s_cols[:, si : si + W_t * 128] = (
                    S.reshape(W_t, 128, 128).transpose(1, 0, 2).reshape(128, W_t * 128)
                )
                t = pr * 2 + hf
                lo2 = c * ROWS_PER_CORE + t * 128
                rows = min(128, ROWS_PER_CORE - t * 128)
                tci = t * N_PATHS + p
                bsc[:rows, tci] = (deg_ins[p][lo2 : lo2 + rows] ** -0.5) * 0.25
        per_core.append({
            "idx": idxw,
            "smat": s_cols.astype(np.float32).astype(F8E4),
            "bscale": bsc,
        })

    shared = {
        **feats,
        "w": Wt,
        "bm": bm_bcast,
        "identity": np.eye(128, dtype=BF16),
    }
    cfg = (
        tuple(int(x) for x in Bc),
        tuple(int(x) for x in We),
        tuple(int(x) for x in Wo),
    )
    return cfg, shared, per_core


def kernel(feat, src, dst, W, b):
    cfg, shared, per_core = _prep_host(feat, src, dst, W, b)
    nc = _build_program(cfg)
    in_maps = [{**shared, **pc} for pc in per_core]
    res = run_bass_kernel_spmd(nc, in_maps, list(range(NCORES)))
    out = np.concatenate([res.results[c]["out"] for c in range(NCORES)], axis=0)
    return out.astype(np.float32)


if __name__ == "__main__":
    rng = np.random.default_rng(0)
    feat = rng.standard_normal((N_NODES, IN_DIM), dtype=np.float32)
    src = rng.integers(0, N_NODES, (N_PATHS, 160000)).astype(np.int64)
    dst = rng.integers(0, N_NODES, (N_PATHS, 160000)).astype(np.int64)
    W = (rng.standard_normal((N_PATHS, IN_DIM, OUT_DIM), dtype=np.float32) / np.sqrt(IN_DIM)).astype(np.float32)
    b = np.zeros((N_PATHS, OUT_DIM), np.float32)
    out = kernel(feat=feat, src=src, dst=dst, W=W, b=b)
    print("kernel ran, out shape", out.shape, out.dtype)


# revision 13
# speedup vs baseline: 1.1602x; 1.0371x over previous
"""MetaPathEncoder (4x GraphConv + mean fusion) as a Bass/Tile SPMD kernel on 8 TRN2 cores.

Strategy (1D dst-node sharding, all 4 metapaths per core):
  - Each core owns 1250 output rows (10000/8), processed as 5 pairs of
    128-row dst tiles. Edges are bucketed on host by (core, path, tile-PAIR)
    and source-deduplicated across the pair: sources feeding both tiles of a
    pair are gathered once (~10% fewer rows than per-tile dedup). Slots are
    laid out [even-only | shared | odd-only] so each tile's scatter matrix
    covers a static contiguous block window.
  - GraphConv norm factorization: deg_out(src)^-1/2 is pre-multiplied into a
    per-path copy of the features (bf16, in HBM); deg_in(dst)^-1/2 * 1/4 is
    applied per dst row by the ACT engine when copying the SpMM result out of
    PSUM. The scatter matrix S[slot, dst_local] holds small-int edge counts,
    shipped as fp8e4 (exact for counts < 16) and fed to the PE directly as
    the stationary operand (no upconvert pass).
  - On device, per (pair, path): dma_gather the slot source rows (bf16) of
    feat_p from HBM, round-robin on SWDGE queues 0-3 by path.
  - Segment-sum via PE matmuls accumulating in fp32 PSUM per tile:
    h[dst, :] = sum_b S_b.T @ X_b over the tile's block window; ACT copies h
    out of PSUM with the per-dst-row scale; PE transposes h (identity matmul)
    to fi-on-partitions; 16 accumulating matmuls per tile apply the four
    512x512 weights: out = sum_p h_p @ W_p + mean(b). The [1250, 512] fp32
    shard is DMA'd out and the host concatenates the 8 shards.
  - Const loads are split across HWDGE queues: idx (gather dependency) first
    on the sync queue, everything else on the scalar queue.
"""
import sys

for _p in ("/opt/trn_rl_repo",):
    if _p not in sys.path:
        sys.path.insert(0, _p)

import numpy as np
import ml_dtypes

import concourse.bass as bass
import concourse.tile as tile
from concourse import bacc, mybir
from concourse.bass_utils import run_bass_kernel_spmd

BF16 = ml_dtypes.bfloat16
F8E4 = ml_dtypes.float8_e4m3fn

N_NODES = 10000
N_PATHS = 4
IN_DIM = 512
OUT_DIM = 512
NCORES = 8
ROWS_PER_CORE = N_NODES // NCORES  # 1250
NTILES = (ROWS_PER_CORE + 127) // 128  # 10 (last tile has 98 rows)
NPAIRS = NTILES // 2  # 5
NCALLS = NPAIRS * N_PATHS  # 20 gather calls per core
NTCALLS = NTILES * N_PATHS  # 40 per-tile compute units

_program_cache: dict[tuple, object] = {}


def _build_program(cfg: tuple):
    """Build the SPMD Bass program.

    cfg = (Bc, We, Wo): per call (pair*4+path): gather blocks, even-tile
    window blocks (from 0), odd-tile window blocks (ending at Bc)."""
    if cfg in _program_cache:
        return _program_cache[cfg]
    Bc, We, Wo = cfg

    TI = sum(Bc) * 8    # idx cols (int16, wrapped 16x, replicated 8x)
    TS = (sum(We) + sum(Wo)) * 128  # S cols (fp8e4), per-tile matrices

    dt = mybir.dt
    nc = bacc.Bacc(
        "TRN2",
        target_bir_lowering=False,
        debug=False,
        num_devices=NCORES,
        num_swdge_queues=4,
    )

    featd = [
        nc.dram_tensor(f"feat{p}", [N_NODES, IN_DIM], dt.bfloat16, kind="ExternalInput").ap()
        for p in range(N_PATHS)
    ]
    idxd = nc.dram_tensor("idx", [128, TI], dt.int16, kind="ExternalInput").ap()
    sd = nc.dram_tensor("smat", [128, TS], dt.float8e4, kind="ExternalInput").ap()
    wd = nc.dram_tensor("w", [128, 16 * OUT_DIM], dt.bfloat16, kind="ExternalInput").ap()
    bmd = nc.dram_tensor("bm", [128, OUT_DIM], dt.float32, kind="ExternalInput").ap()
    bsd = nc.dram_tensor("bscale", [128, NTCALLS], dt.float32, kind="ExternalInput").ap()
    identd = nc.dram_tensor("identity", [128, 128], dt.bfloat16, kind="ExternalInput").ap()
    outd = nc.dram_tensor("out", [ROWS_PER_CORE, OUT_DIM], dt.float32, kind="ExternalOutput").ap()

    off_i = [0]
    for bq in Bc:
        off_i.append(off_i[-1] + bq * 8)
    # S offsets: per (call, half)
    off_s = [0]
    for c in range(NCALLS):
        off_s.append(off_s[-1] + We[c] * 128)
        off_s.append(off_s[-1] + Wo[c] * 128)

    with tile.TileContext(nc) as tc:
        with (
            tc.tile_pool(name="const", bufs=1) as cpool,
            tc.tile_pool(name="g", bufs=8) as gpool,
            tc.tile_pool(name="s8", bufs=6) as s8pool,
            tc.tile_pool(name="hsb", bufs=3) as hsb_pool,
            tc.tile_pool(name="htsb", bufs=3) as htsb_pool,
            tc.tile_pool(name="osb", bufs=2) as osb_pool,
            tc.tile_pool(name="hps", bufs=2, space="PSUM") as hps_pool,
            tc.tile_pool(name="htps", bufs=2, space="PSUM") as htps_pool,
            tc.tile_pool(name="ops", bufs=2, space="PSUM") as ops_pool,
        ):
            # idx first (gathers depend on it); first pair's columns, then rest
            idx_sb = cpool.tile([128, TI], dt.int16)
            split = off_i[N_PATHS]
            nc.sync.dma_start(idx_sb[:, :split], idxd[:, :split])
            nc.sync.dma_start(idx_sb[:, split:], idxd[:, split:])
            # remaining consts on the scalar HWDGE queue (parallel with idx)
            bs_sb = cpool.tile([128, NTCALLS], dt.float32)
            nc.scalar.dma_start(bs_sb[:], bsd[:])
            ident = cpool.tile([128, 128], dt.bfloat16)
            nc.scalar.dma_start(ident[:], identd[:])
            w_sb = cpool.tile([128, 16 * OUT_DIM], dt.bfloat16)
            nc.scalar.dma_start(w_sb[:], wd[:])
            bm_sb = cpool.tile([128, OUT_DIM], dt.float32)
            nc.scalar.dma_start(bm_sb[:], bmd[:])

            for pair in range(NPAIRS):
                out_pse = ops_pool.tile([128, OUT_DIM], dt.float32, name="out_pse")
                out_pso = ops_pool.tile([128, OUT_DIM], dt.float32, name="out_pso")
                for p in range(N_PATHS):
                    call = pair * N_PATHS + p
                    B = Bc[call]
                    We_c, Wo_c = We[call], Wo[call]
                    Bb = B - We_c  # second-half blocks (may be 0)
                    # split gather: blocks [0, We) and [We, B) as separate
                    # instructions so they pipeline like small gathers
                    ga = gpool.tile([128, We_c, IN_DIM], dt.bfloat16, name="g")
                    nc.gpsimd.dma_gather(
                        ga[:],
                        featd[p][:],
                        idx_sb[:, off_i[call] : off_i[call] + We_c * 8],
                        We_c * 128,
                        We_c * 128,
                        IN_DIM,
                        single_packet=False,
                        queue_num=(2 * call) % 4,
                    )
                    if Bb > 0:
                        gb = gpool.tile([128, Bb, IN_DIM], dt.bfloat16, name="g")
                        nc.gpsimd.dma_gather(
                            gb[:],
                            featd[p][:],
                            idx_sb[:, off_i[call] + We_c * 8 : off_i[call + 1]],
                            Bb * 128,
                            Bb * 128,
                            IN_DIM,
                            single_packet=False,
                            queue_num=(2 * call + 1) % 4,
                        )
                    else:
                        gb = None
                    for half, out_ps in ((0, out_pse), (1, out_pso)):
                        t = pair * 2 + half
                        W_t = We_c if half == 0 else Wo_c
                        blo = 0 if half == 0 else B - W_t
                        si = off_s[call * 2 + half]
                        s8 = s8pool.tile([128, W_t, 128], dt.float8e4, name="s8")
                        nc.sync.dma_start(
                            s8[:].rearrange("p b d -> p (b d)"),
                            sd[:, si : si + W_t * 128],
                        )
                        hp = hps_pool.tile([128, IN_DIM], dt.float32, name="hp")
                        for bb in range(W_t):
                            gblk = blo + bb
                            gsrc = ga[:, gblk, :] if gblk < We_c else gb[:, gblk - We_c, :]
                            nc.tensor.matmul(
                                hp[:],
                                s8[:, bb, :],
                                gsrc,
                                start=(bb == 0),
                                stop=(bb == W_t - 1),
                            )
                        tci = t * N_PATHS + p
                        hs = hsb_pool.tile([128, IN_DIM], dt.bfloat16, name="hs")
                        nc.scalar.mul(hs[:], hp[:], bs_sb[:, tci : tci + 1])
                        htp = htps_pool.tile([128, IN_DIM], dt.bfloat16, name="htp")
                        for cc in range(4):
                            nc.tensor.transpose(
                                htp[:, cc * 128 : (cc + 1) * 128],
                                hs[:, cc * 128 : (cc + 1) * 128],
                                ident[:],
                            )
                        hts = htsb_pool.tile([128, IN_DIM], dt.bfloat16, name="hts")
                        nc.vector.tensor_copy(hts[:], htp[:])
                        for cc in range(4):
                            nc.tensor.matmul(
                                out_ps[:],
                                hts[:, cc * 128 : (cc + 1) * 128],
                                w_sb[:, (p * 4 + cc) * OUT_DIM : (p * 4 + cc + 1) * OUT_DIM],
                                start=(p == 0 and cc == 0),
                                stop=(p == N_PATHS - 1 and cc == 3),
                            )
                for half, out_ps in ((0, out_pse), (1, out_pso)):
                    t = pair * 2 + half
                    os_ = osb_pool.tile([128, OUT_DIM], dt.float32, name="os_")
                    nc.vector.tensor_add(os_[:], out_ps[:], bm_sb[:])
                    rows = min(128, ROWS_PER_CORE - t * 128)
                    nc.sync.dma_start(outd[t * 128 : t * 128 + rows, :], os_[:rows, :])

    nc.compile()
    _program_cache[cfg] = nc
    return nc


def _prep_host(feat, src, dst, W, b):
    """Host-side pair bucketing, cross-tile dedup, windowed slot layout.

    Returns (cfg tuple, shared dict, per-core dicts)."""
    src = np.asarray(src).astype(np.int64)
    dst = np.asarray(dst).astype(np.int64)
    feat = np.asarray(feat, dtype=np.float32)
    W = np.asarray(W, dtype=np.float32)
    b = np.asarray(b, dtype=np.float32)

    Wt = np.empty((128, 16, OUT_DIM), dtype=BF16)
    for p in range(N_PATHS):
        for c in range(4):
            Wt[:, p * 4 + c, :] = W[p, c * 128 : (c + 1) * 128, :].astype(BF16)
    Wt = np.ascontiguousarray(Wt.reshape(128, 16 * OUT_DIM))

    bmean = b.mean(0).astype(np.float32)
    bm_bcast = np.ascontiguousarray(np.broadcast_to(bmean, (128, OUT_DIM)))

    feats = {}
    deg_ins = []
    sorted_data = []
    for p in range(N_PATHS):
        s, d = src[p], dst[p]
        deg_out = np.maximum(np.bincount(s, minlength=N_NODES), 1).astype(np.float64)
        deg_in = np.maximum(np.bincount(d, minlength=N_NODES), 1).astype(np.float64)
        feats[f"feat{p}"] = (feat * (deg_out**-0.5)[:, None]).astype(BF16)
        deg_ins.append(deg_in)
        order = np.argsort(d, kind="stable")
        sorted_data.append((s[order], d[order]))

    # pair bounds: rows [pair*256, min(pair*256+256, 1250)) per core
    los, his = [], []
    for c in range(NCORES):
        base = c * ROWS_PER_CORE
        for pr in range(NPAIRS):
            los.append(base + pr * 256)
            his.append(base + min((pr + 1) * 256, ROWS_PER_CORE))
    los, his = np.array(los), np.array(his)
    ranges = []
    for p in range(N_PATHS):
        ds = sorted_data[p][1]
        a = np.searchsorted(ds, los, side="left")
        e = np.searchsorted(ds, his, side="left")
        ranges.append((a, e))

    # per (core, pair, path): slots = unique sources of the pair;
    # categorize even-only / shared / odd-only
    buckets = {}
    n_cat = {}
    Bc = np.zeros(NCALLS, dtype=np.int64)
    for c in range(NCORES):
        for pr in range(NPAIRS):
            lo = c * ROWS_PER_CORE + pr * 256
            for p in range(N_PATHS):
                call = pr * N_PATHS + p
                a, e = ranges[p][0][c * NPAIRS + pr], ranges[p][1][c * NPAIRS + pr]
                ss = sorted_data[p][0][a:e]
                dl = (sorted_data[p][1][a:e] - lo).astype(np.int64)  # 0..255
                half = dl // 128
                uniq, inv = np.unique(ss, return_inverse=True)
                U = len(uniq)
                in_e = np.zeros(U, bool)
                in_o = np.zeros(U, bool)
                in_e[inv[half == 0]] = True
                in_o[inv[half == 1]] = True
                cat = np.where(in_e & in_o, 1, np.where(in_e, 0, 2))
                order2 = np.argsort(cat, kind="stable")
                uniq_s = uniq[order2]
                rank = np.empty(U, np.int64)
                rank[order2] = np.arange(U)
                slot_of_edge = rank[inv]
                n_e = int((cat == 0).sum())
                n_sh = int((cat == 1).sum())
                n_o = int((cat == 2).sum())
                buckets[(c, call)] = (uniq_s, slot_of_edge, dl, half, n_e, n_sh, n_o)
                n_cat[(c, call)] = (n_e, n_sh, n_o)
                Bc[call] = max(Bc[call], (U + 127) // 128)
    Bc = np.maximum(Bc, 1)

    # static windows: even tile covers blocks [0, We); odd covers [B-Wo, B)
    We = np.zeros(NCALLS, dtype=np.int64)
    Wo = np.zeros(NCALLS, dtype=np.int64)
    for call in range(NCALLS):
        B = int(Bc[call])
        for c in range(NCORES):
            n_e, n_sh, n_o = n_cat[(c, call)]
            We[call] = max(We[call], (n_e + n_sh + 127) // 128)
            Wo[call] = max(Wo[call], (n_sh + n_o + 127) // 128)
        We[call] = max(1, min(int(We[call]), B))
        Wo[call] = max(1, min(int(Wo[call]), B))
        # the window overlap [B-Wo, We) must hold every core's shared slots
        need = max((n_cat[(c, call)][1] + 127) // 128 for c in range(NCORES))
        if int(We[call]) + int(Wo[call]) - B < need:
            We[call] = min(B, B + need - int(Wo[call]))

    off_i = np.concatenate([[0], np.cumsum(Bc * 8)])
    off_s = [0]
    for call in range(NCALLS):
        off_s.append(off_s[-1] + int(We[call]) * 128)
        off_s.append(off_s[-1] + int(Wo[call]) * 128)
    TI, TS = int(off_i[-1]), int(off_s[-1])

    per_core = []
    for c in range(NCORES):
        idxw = np.zeros((128, TI), dtype=np.int16)
        s_cols = np.zeros((128, TS), dtype=np.uint8)
        bsc = np.zeros((128, NTCALLS), dtype=np.float32)
        for call in range(NCALLS):
            B = int(Bc[call])
            pr, p = call // N_PATHS, call % N_PATHS
            uniq_s, slot_of_edge, dl, half, n_e, n_sh, n_o = buckets[(c, call)]
            U = len(uniq_s)
            # place slots: even-only at 0; shared inside both windows;
            # odd-only at the end. Shared must sit in [B-Wo, We)*128.
            sh_lo_min = (B - int(Wo[call])) * 128
            sh_start = max(n_e, sh_lo_min)
            assert sh_start + n_sh <= int(We[call]) * 128, (call, c)
            o_start = B * 128 - n_o
            assert o_start >= sh_start + n_sh, (call, c)
            pos = np.empty(U, np.int64)
            pos[:n_e] = np.arange(n_e)
            pos[n_e : n_e + n_sh] = sh_start + np.arange(n_sh)
            pos[n_e + n_sh :] = o_start + np.arange(n_o)
            idx_pad = np.zeros(B * 128, dtype=np.int16)
            idx_pad[pos] = uniq_s
            w16 = idx_pad.reshape(B * 8, 16).T
            idxw[:, off_i[call] : off_i[call + 1]] = np.tile(w16, (8, 1))
            # per-tile S over its window
            slot_pos_of_edge = pos[slot_of_edge]
            for hf in range(2):
                W_t = int(We[call]) if hf == 0 else int(Wo[call])
                blo = 0 if hf == 0 else B - W_t
                m = half == hf
                sp = slot_pos_of_edge[m] - blo * 128
                dloc = dl[m] - hf * 128
                S = np.zeros((W_t * 128, 128), dtype=np.int64)
                np.add.at(S, (sp, dloc), 1)
                assert S.max() < 16
                si = off_s[call * 2 + hf]
                s_cols[:, si : si + W_t * 128] = (
                    S.reshape(W_t, 128, 128).transpose(1, 0, 2).reshape(128, W_t * 128)
                )
                t = pr * 2 + hf
                lo2 = c * ROWS_PER_CORE + t * 128
                rows = min(128, ROWS_PER_CORE - t * 128)
                tci = t * N_PATHS + p
                bsc[:rows, tci] = (deg_ins[p][lo2 : lo2 + rows] ** -0.5) * 0.25
        per_core.append({
            "idx": idxw,
            "smat": s_cols.astype(np.float32).astype(F8E4),
            "bscale": bsc,
        })

    shared = {
        **feats,
        "w": Wt,
        "bm": bm_bcast,
        "identity": np.eye(128, dtype=BF16),
    }
    cfg = (
        tuple(int(x) for x in Bc),
        tuple(int(x) for x in We),
        tuple(int(x) for x in Wo),
    )
    return cfg, shared, per_core


def kernel(feat, src, dst, W, b):
    cfg, shared, per_core = _prep_host(feat, src, dst, W, b)
    nc = _build_program(cfg)
    in_maps = [{**shared, **pc} for pc in per_core]
    res = run_bass_kernel_spmd(nc, in_maps, list(range(NCORES)))
    out = np.concatenate([res.results[c]["out"] for c in range(NCORES)], axis=0)
    return out.astype(np.float32)


if __name__ == "__main__":
    rng = np.random.default_rng(0)
    feat = rng.standard_normal((N_NODES, IN_DIM), dtype=np.float32)
    src = rng.integers(0, N_NODES, (N_PATHS, 160000)).astype(np.int64)
    dst = rng.integers(0, N_NODES, (N_PATHS, 160000)).astype(np.int64)
    W = (rng.standard_normal((N_PATHS, IN_DIM, OUT_DIM), dtype=np.float32) / np.sqrt(IN_DIM)).astype(np.float32)
    b = np.zeros((N_PATHS, OUT_DIM), np.float32)
    out = kernel(feat=feat, src=src, dst=dst, W=W, b=b)
    print("kernel ran, out shape", out.shape, out.dtype)
